# revision 24
# baseline (speedup 1.0000x reference)
"""2-layer GAT (nn_GATModel) on 8 Trainium2 NeuronCores — v3.

Strategy (v3)
-------------
The critical resource is SWDGE descriptor generation on Q7 cores 0-1
(~7.9 ns/gathered row, independent of row size).  v3 keeps that stream
running continuously and hides everything else behind it:

- NO T1 table / phase A: layer-1 gathers fetch raw x rows (256B, bf16)
  with transpose=True, so gathered data lands feature-major
  [128 x-feat, edges].  A per-chunk PE matmul (xgT_c @ [W1|vsrc])
  produces edge-major h1 rows AND per-edge a_src directly in PSUM —
  the transpose is free, and layer-1 gathers have NO data deps, so
  desc-gen starts at t=0.
- Attention logits accumulate in PSUM: alpha[:,c,0:8] = seT_c^T@adst1
  (+) xgT_c^T@vsrc, then one scalar-engine Lrelu+Exp per group.
- idx16/dcol loaded once in SBUF (identical for both layers).
- Vector diet: one-hot builds kept, per-tile epilogues fused/minimal,
  Lrelu/Exp/PSUM casts on the Scalar engine, self-loop weights
  (wself/w2s/uself2) precomputed once for all 49 tiles.
- Phase D (layer 2) as before: non-transpose 256B gathers of the
  AllGathered T2 table; epilogue batched via precomputed w2s/uself2.
"""

import sys

sys.path.insert(0, "/opt/trn_rl_repo")

import heapq
import math
from contextlib import ExitStack

import numpy as np
import ml_dtypes

import concourse.bass as bass  # noqa: F401
import concourse.tile as tile
from concourse import bacc, mybir
from concourse.bass_utils import run_bass_kernel_spmd
from concourse.masks import make_identity

P = 128
NEG_SLOPE = 0.2
F32 = mybir.dt.float32
BF16 = mybir.dt.bfloat16
I16 = mybir.dt.int16
BF = ml_dtypes.bfloat16

REAL = dict(
    n=50000, ncore=8, half=32768, heads=8, cdim=32, in_ch=128,
    cap_chunks=40, row2=128,
)


# ---------------------------------------------------------------- host prep
def assign_nodes(deg, n, ncore, sh):
    """LPT-balance nodes into (core, tile) bins; return nodes_at, pos_of."""
    ntile = math.ceil(sh / P)
    caps = [P] * (ntile - 1) + [sh - P * (ntile - 1)]
    nbins = ncore * ntile
    order = np.argsort(-deg, kind="stable")
    heap = [(0, b) for b in range(nbins)]
    heapq.heapify(heap)
    fill = [[] for _ in range(nbins)]
    for v in order:
        while True:
            s, b = heapq.heappop(heap)
            if len(fill[b]) < caps[b % ntile]:
                break
        fill[b].append(v)
        if len(fill[b]) < caps[b % ntile]:
            heapq.heappush(heap, (s + int(deg[v]), b))
    nodes_at = np.empty(n, np.int64)
    pos_of = np.empty(n, np.int64)
    for b in range(nbins):
        k, t = divmod(b, ntile)
        base = k * sh + t * P
        for i, v in enumerate(fill[b]):
            nodes_at[base + i] = v
            pos_of[v] = base + i
    return nodes_at, pos_of, ntile


def build_schedule(src, dst, p):
    n, ncore, half = p["n"], p["ncore"], p["half"]
    sh = n // ncore
    deg = np.bincount(dst, minlength=n)
    nodes_at, pos_of, ntile = assign_nodes(deg, n, ncore, sh)

    sp = pos_of[src]
    dp = pos_of[dst]
    core = dp // sh
    within = dp % sh
    tl = within >> 7
    dloc = within & 127
    hi = (sp >= half).astype(np.int64)
    key = ((core * ntile + tl) * 2 + hi)
    order = np.argsort(key, kind="stable")
    sp, key = sp[order], key[order]
    dloc_s = dloc[order]

    bounds = np.searchsorted(key, np.arange(ncore * ntile * 2 + 1))
    counts = (bounds[1:] - bounds[:-1]).reshape(ncore, ntile, 2)
    cmax = counts.max(axis=0)                       # [ntile, 2]
    csched = np.maximum(-(-cmax // P), 1)

    groups = []
    cur = []
    for t in range(ntile):
        ct = int(csched[t].sum())
        if cur and sum(int(csched[x].sum()) for x in cur) + ct > p["cap_chunks"]:
            groups.append(cur)
            cur = []
        cur.append(t)
    groups.append(cur)

    chunk_tile = []
    group_info = []
    tile_info = {t: {} for t in range(ntile)}
    c0 = 0
    for g in groups:
        glow = sum(int(csched[t][0]) for t in g)
        ghigh = sum(int(csched[t][1]) for t in g)
        gi = dict(tiles=g, c0=c0, clow=glow, chigh=ghigh, ct=glow + ghigh)
        cc = c0
        for half_id in (0, 1):
            for t in g:
                nch = int(csched[t][half_id])
                tile_info[t]["low" if half_id == 0 else "high"] = (cc, nch)
                for _ in range(nch):
                    chunk_tile.append(t)
                cc += nch
        c0 = cc
        group_info.append(gi)
    nchunks = c0

    sched = dict(
        sh=sh, ntile=ntile, csched=csched, groups=group_info,
        chunk_tile=np.array(chunk_tile), tile_info=tile_info, nchunks=nchunks,
        tile_rows=[min(P, sh - t * P) for t in range(ntile)],
    )

    percore = []
    for k in range(ncore):
        idxs = np.zeros(nchunks * P, np.int16)
        dla = np.full(nchunks * P, 999.0, np.float32)
        for t in range(ntile):
            for half_id, kkey in ((0, "low"), (1, "high")):
                b0 = bounds[(k * ntile + t) * 2 + half_id]
                b1 = bounds[(k * ntile + t) * 2 + half_id + 1]
                vals = sp[b0:b1] - (half if half_id else 0)
                dv = dloc_s[b0:b1]
                cstart, cnum = tile_info[t][kkey]
                s0 = cstart * P
                idxs[s0 : s0 + len(vals)] = vals.astype(np.int16)
                dla[s0 : s0 + len(vals)] = dv.astype(np.float32)
        idx16 = np.tile(np.ascontiguousarray(idxs.reshape(-1, 16).T), (8, 1))
        dstcol = np.ascontiguousarray(
            dla.reshape(nchunks, P).T.astype(np.float32))
        dstrow = dla.reshape(1, -1).astype(BF)
        percore.append(dict(idx16=idx16, dstcol=dstcol, dstrow=dstrow))
    return sched, percore, nodes_at, pos_of


def prep_weights(W1, att_src1, att_dst1, bias1, W2, att_src2, att_dst2, p):
    H, C = p["heads"], p["cdim"]
    vsrc1 = np.einsum("khc,hc->kh", W1.reshape(-1, H, C), att_src1)
    vdst1 = np.einsum("khc,hc->kh", W1.reshape(-1, H, C), att_dst1)
    w1ev = np.concatenate([W1, vsrc1, vdst1], axis=1)       # [128, 272]

    vsrc2 = W2 @ att_src2[0]
    vdst2 = W2 @ att_dst2[0]
    r2ext = np.concatenate(
        [W2, vsrc2[:, None], vdst2[:, None]], axis=1)        # [256, 34]
    r2b = r2ext.astype(BF)
    # csum must match the QUANTIZED weights: the elu+1 encoding relies on
    # exact cancellation of the +1 term (sum over 256 rows).
    csum = r2b.astype(np.float32).sum(axis=0)
    return (w1ev.astype(BF), r2b,
            csum.astype(np.float32)[None, :],
            bias1.astype(np.float32)[None, :])


# ---------------------------------------------------------------- device
def build_program(sched, p):
    n, ncore, half = p["n"], p["ncore"], p["half"]
    H, C, IC = p["heads"], p["cdim"], p["in_ch"]
    HC = H * C                                      # 256
    ROW2 = p["row2"]                                # 128 (bf16 units)
    R2E = C + 2                                     # 34
    sh, ntile, nchunks = sched["sh"], sched["ntile"], sched["nchunks"]
    cap = p["cap_chunks"]
    natile = math.ceil(n / P)                       # 391
    npos = natile * P                               # 50048

    nc = bacc.Bacc("TRN2", target_bir_lowering=False, debug=False,
                   num_devices=ncore)

    i_xpos = nc.dram_tensor("xpos", [npos, IC], BF16, kind="ExternalInput")
    i_xTmy = nc.dram_tensor("xTmy", [IC, sh], BF16, kind="ExternalInput")
    i_idx = nc.dram_tensor("idx16", [P, nchunks * 8], I16, kind="ExternalInput")
    i_dstcol = nc.dram_tensor("dstcol", [P, nchunks], F32, kind="ExternalInput")
    i_dstrow = nc.dram_tensor("dstrow", [1, nchunks * P], BF16,
                              kind="ExternalInput")
    i_w1ev = nc.dram_tensor("w1ev", [IC, HC + 2 * H], BF16,
                            kind="ExternalInput")
    i_r2 = nc.dram_tensor("r2ext", [HC, R2E], BF16, kind="ExternalInput")
    i_csum = nc.dram_tensor("csum", [1, R2E], F32, kind="ExternalInput")
    i_b1 = nc.dram_tensor("bias1", [1, HC], F32, kind="ExternalInput")
    i_b2 = nc.dram_tensor("bias2", [1, C], F32, kind="ExternalInput")
    o_out = nc.dram_tensor("out", [sh, C], F32, kind="ExternalOutput")

    d_t2s = nc.dram_tensor("T2slab", [sh, ROW2], BF16)
    d_t2f = nc.dram_tensor("T2full", [n, ROW2], BF16,
                           addr_space="Shared" if ncore > 4 else "Local")

    AF = mybir.ActivationFunctionType
    OP = mybir.AluOpType

    with tile.TileContext(nc) as tc, ExitStack() as ctx:
        const = ctx.enter_context(tc.tile_pool(name="const", bufs=1))

        iota_col = const.tile([P, 1], F32)
        nc.gpsimd.iota(iota_col[:], pattern=[[0, 1]], base=0,
                       channel_multiplier=1,
                       allow_small_or_imprecise_dtypes=True)
        iota_rep = const.tile([P, cap * P], BF16)
        nc.gpsimd.iota(iota_rep[:], pattern=[[0, cap], [1, P]], base=0,
                       channel_multiplier=0,
                       allow_small_or_imprecise_dtypes=True)
        identbf = const.tile([P, P], BF16)
        make_identity(nc, identbf[:])
        w1ev_sb = const.tile([IC, HC + 2 * H], BF16)
        nc.sync.dma_start(w1ev_sb[:], i_w1ev[:, :])
        r2_sb = const.tile([P, 2, R2E], BF16)
        for j in range(2):
            nc.sync.dma_start(r2_sb[:, j, :], i_r2[j * P : (j + 1) * P, :])
        csum_sb = const.tile([P, R2E], F32)
        nc.gpsimd.dma_start(csum_sb[:], i_csum[:, :].to_broadcast([P, R2E]))
        b1_sb = const.tile([P, HC], F32)
        nc.gpsimd.dma_start(b1_sb[:], i_b1[:, :].to_broadcast([P, HC]))
        b2_sb = const.tile([P, C], F32)
        nc.gpsimd.dma_start(b2_sb[:], i_b2[:, :].to_broadcast([P, C]))
        xTmy_sb = const.tile([IC, sh], BF16)
        nc.sync.dma_start(xTmy_sb[:], i_xTmy[:, :])
        adst1_sb = const.tile([P, ntile, H], BF16)
        nc.vector.memset(adst1_sb[:], 0.0)
        h1own_sb = const.tile([P, ntile, HC + 2 * H], BF16)
        nc.vector.memset(h1own_sb[:], 0.0)
        wself_sb = const.tile([P, ntile, H], F32)
        adst2e_sb = const.tile([P, nchunks], F32)
        t2keep = const.tile([P, ntile, R2E], F32)
        w2s_sb = const.tile([P, ntile], F32)
        uself2_sb = const.tile([P, ntile, C], F32)

        # warm-up gather: load the Q7 gather ucode before the real stream
        with tc.tile_pool(name="warm", bufs=1) as wrm:
            widx = wrm.tile([P, 8], I16)
            nc.gpsimd.memset(widx[:], 0)
            wout = wrm.tile([P, 1, P], BF16)
            nc.gpsimd.dma_gather(
                out_ap=wout[:], in_ap=i_xpos[0:P, :], idxs_ap=widx[:],
                num_idxs=P, num_idxs_reg=P, elem_size=IC,
                transpose=True, single_packet=False)

        # ---- phase X: my nodes' h1 rows + a_dst1 (from xTmy) ----
        with tc.tile_pool(name="xpp", bufs=2, space="PSUM") as xpp:
            for t in range(ntile):
                rows = sched["tile_rows"][t]
                hp = xpp.tile([P, HC + 2 * H], F32, tag="hp")
                nc.tensor.matmul(out=hp[:rows, :],
                                 lhsT=xTmy_sb[:, t * P : t * P + rows],
                                 rhs=w1ev_sb[:], start=True, stop=True)
                nc.scalar.copy(h1own_sb[:rows, t, 0:HC], hp[:rows, 0:HC])
                nc.vector.tensor_copy(
                    h1own_sb[:rows, t, HC : HC + 2 * H].bitcast(F32),
                    hp[:rows, HC : HC + H])
                nc.vector.tensor_copy(adst1_sb[:rows, t, :],
                                      hp[:rows, HC + H : HC + 2 * H])

        # wself = exp(lrelu(a_src_own + a_dst_own)) for all tiles at once
        with tc.tile_pool(name="wsp", bufs=1) as wsp:
            aown = wsp.tile([P, ntile, H], F32)
            nc.vector.tensor_tensor(
                out=aown[:],
                in0=h1own_sb[:, :, HC : HC + 2 * H].bitcast(F32),
                in1=adst1_sb[:, :, :], op=OP.add)
            nc.scalar.activation(aown[:], aown[:], AF.Prelu, alpha=NEG_SLOPE)
            nc.scalar.activation(wself_sb[:], aown[:], AF.Exp)

        # ---- phase B: layer-1 gather + aggregate + dense layer 2 ----
        # 3-stage software pipeline so the PE queue never embeds waits on
        # the vector/scalar epilogue chain:
        #   stage1(i): gathers + one-hots + per-chunk transform of group i
        #   stage2(i): aggregation matmuls + softmax/elu epilogue (V/S)
        #   stage3(i): L2-transform PE work (transpose/h2/col8) + writes
        with tc.tile_pool(name="xg", bufs=3) as xgp, \
             tc.tile_pool(name="gw", bufs=2) as gwp, \
             tc.tile_pool(name="bse", bufs=3) as bsp, \
             tc.tile_pool(name="sea", bufs=3) as sep, \
             tc.tile_pool(name="sc1", bufs=3) as scp, \
             tc.tile_pool(name="ubp", bufs=6) as ubp, \
             tc.tile_pool(name="hp1", bufs=2, space="PSUM") as hpp, \
             tc.tile_pool(name="al1", bufs=2, space="PSUM") as alp, \
             tc.tile_pool(name="agg1", bufs=2, space="PSUM") as aggp, \
             tc.tile_pool(name="tp1", bufs=1, space="PSUM") as tpp, \
             tc.tile_pool(name="h2p", bufs=1, space="PSUM") as h2pp:

            def b_stage1(gi):
                c0, cl, chg, ct = gi["c0"], gi["clow"], gi["chigh"], gi["ct"]
                s0 = c0 * P
                idx_t = bsp.tile([P, ct * 8], I16, tag="idx")
                nc.sync.dma_start(idx_t[:], i_idx[:, c0 * 8 : (c0 + ct) * 8])
                dcol_t = bsp.tile([P, ct], F32, tag="dcol")
                nc.sync.dma_start(dcol_t[:], i_dstcol[:, c0 : c0 + ct])
                xgT = xgp.tile([P, 1, cap * P], BF16, tag="xgT")
                for (off, num, b0, b1_) in ((0, cl, 0, half),
                                            (cl, chg, half, npos)):
                    if num == 0:
                        continue
                    nc.gpsimd.dma_gather(
                        out_ap=xgT[:, :, off * P : (off + num) * P],
                        in_ap=i_xpos[b0:b1_, :],
                        idxs_ap=idx_t[:, off * 8 : (off + num) * 8],
                        num_idxs=num * P, num_idxs_reg=num * P,
                        elem_size=IC, transpose=True, single_packet=False)
                # bc AFTER the gathers: it must not gate the gather stream
                bc = bsp.tile([P, ct * P], BF16, tag="bc")
                nc.gpsimd.dma_start(
                    bc[:], i_dstrow[:, s0 : s0 + ct * P].to_broadcast(
                        [P, ct * P]))
                # seT built in place over bc
                nc.vector.tensor_tensor(
                    out=bc[:], in0=bc[:],
                    in1=iota_col[:, 0:1].to_broadcast([P, ct * P]),
                    op=OP.is_equal)
                seT = bc
                seA = sep.tile([P, ct * P], BF16, tag="seA")
                nc.vector.tensor_tensor(
                    out=seA[:].rearrange("p (c q) -> p c q", q=P),
                    in0=iota_rep[:, 0 : ct * P].rearrange(
                        "p (c q) -> p c q", q=P),
                    in1=dcol_t[:].rearrange(
                        "p (c o) -> p c o", o=1).to_broadcast([P, ct, P]),
                    op=OP.is_equal)
                Gw = gwp.tile([P, ct, HC + H], BF16, tag="Gw")
                al_ps = alp.tile([P, ct * H], F32, tag="al")
                alv = al_ps[:].rearrange("p (c z) -> p c z", z=H)
                for c in range(ct):
                    t = sched["chunk_tile"][c0 + c]
                    hp = hpp.tile([P, HC], F32, tag="hp")
                    nc.tensor.matmul(
                        out=hp[:], lhsT=xgT[:, 0, c * P : (c + 1) * P],
                        rhs=w1ev_sb[:, 0:HC], start=True, stop=True)
                    nc.tensor.matmul(
                        out=alv[:, c, :],
                        lhsT=seT[:, c * P : (c + 1) * P],
                        rhs=adst1_sb[:, t, :], start=True, stop=False)
                    nc.tensor.matmul(
                        out=alv[:, c, :],
                        lhsT=xgT[:, 0, c * P : (c + 1) * P],
                        rhs=w1ev_sb[:, HC : HC + H], start=False, stop=True)
                    nc.scalar.copy(Gw[:, c, 0:HC], hp[:])
                asb = scp.tile([P, ct * H], F32, tag="asb")
                nc.scalar.activation(
                    asb[:].rearrange("p (c h) -> p c h", h=H),
                    alv[:, :, :], AF.Prelu, alpha=NEG_SLOPE)
                nc.scalar.activation(
                    Gw[:, :, HC : HC + H],
                    asb[:].rearrange("p (c h) -> p c h", h=H), AF.Exp)
                g4 = Gw[:, :, 0:HC].rearrange("p c (h f) -> p c h f", f=C)
                wb = Gw[:, :, HC : HC + H].rearrange(
                    "p c (h o) -> p c h o", o=1).to_broadcast([P, ct, H, C])
                nc.vector.tensor_tensor(out=g4, in0=g4, in1=wb, op=OP.mult)
                return dict(gi=gi, seT=seT, seA=seA, Gw=Gw)

            def tchunks_of(gi, t):
                c0 = gi["c0"]
                tl, tlc = sched["tile_info"][t]["low"]
                th, thc = sched["tile_info"][t]["high"]
                return [tl - c0 + i_ for i_ in range(tlc)] + \
                       [th - c0 + i_ for i_ in range(thc)]

            def b_stage2(S):
                gi, seA, Gw = S["gi"], S["seA"], S["Gw"]
                S["ub"] = {}
                for t in gi["tiles"]:
                    agg = aggp.tile([P, HC + H], F32, tag="agg")
                    tchunks = tchunks_of(gi, t)
                    for j, c in enumerate(tchunks):
                        nc.tensor.matmul(
                            out=agg[:], lhsT=seA[:, c * P : (c + 1) * P],
                            rhs=Gw[:, c, 0 : HC + H],
                            start=(j == 0), stop=(j == len(tchunks) - 1))
                    dn = scp.tile([P, H], F32, tag="dn")
                    nc.vector.tensor_tensor(out=dn[:], in0=agg[:, HC : HC + H],
                                            in1=wself_sb[:, t, :], op=OP.add)
                    rc = scp.tile([P, H], F32, tag="rc")
                    nc.vector.reciprocal(rc[:], dn[:])
                    u = scp.tile([P, HC], F32, tag="u")
                    nc.vector.tensor_tensor(
                        out=u[:].rearrange("p (h f) -> p h f", f=C),
                        in0=h1own_sb[:, t, 0:HC].rearrange(
                            "p (h f) -> p h f", f=C),
                        in1=wself_sb[:, t, :].rearrange("p (h o) -> p h o", o=1)
                            .to_broadcast([P, H, C]),
                        op=OP.mult)
                    nc.vector.tensor_tensor(out=u[:], in0=u[:],
                                            in1=agg[:, 0:HC], op=OP.add)
                    nc.vector.tensor_tensor(
                        out=u[:].rearrange("p (h f) -> p h f", f=C),
                        in0=u[:].rearrange("p (h f) -> p h f", f=C),
                        in1=rc[:].rearrange("p (h o) -> p h o", o=1)
                            .to_broadcast([P, H, C]),
                        op=OP.mult)
                    nc.vector.tensor_tensor(out=u[:], in0=u[:], in1=b1_sb[:],
                                            op=OP.add)
                    # elu(u)+1 = relu(u) + min(exp(u), 1)
                    eu = scp.tile([P, HC], F32, tag="eu")
                    nc.scalar.activation(eu[:], u[:], AF.Exp)
                    nc.vector.tensor_scalar(out=eu[:], in0=eu[:], scalar1=1.0,
                                            scalar2=None, op0=OP.min)
                    nc.scalar.activation(u[:], u[:], AF.Relu)
                    ub = ubp.tile([P, HC], BF16, tag="ub")
                    nc.vector.tensor_tensor(out=ub[:], in0=u[:], in1=eu[:],
                                            op=OP.add)
                    S["ub"][t] = ub

            def b_stage3(S):
                gi, seT = S["gi"], S["seT"]
                c0, ct = gi["c0"], gi["ct"]
                h2x = h2pp.tile([P, R2E + cap], F32, tag="h2x")
                h2b = h2x[:, R2E : R2E + cap]
                for t in gi["tiles"]:
                    rows = sched["tile_rows"][t]
                    ub = S["ub"][t]
                    h2 = h2x[:, 0:R2E]
                    tp = tpp.tile([P, 2, P], BF16, tag="tp")
                    for j in range(2):
                        nc.tensor.matmul(out=tp[:, j, :],
                                         lhsT=ub[:, j * P : (j + 1) * P],
                                         rhs=identbf[:], is_transpose=True,
                                         start=True, stop=True)
                    uT = scp.tile([P, 2, P], BF16, tag="uT")
                    nc.scalar.copy(uT[:], tp[:])
                    for j in range(2):
                        nc.tensor.matmul(out=h2[:], lhsT=uT[:, j, :],
                                         rhs=r2_sb[:, j, :],
                                         start=(j == 0), stop=(j == 1))
                    nc.vector.tensor_tensor(out=t2keep[:, t, :], in0=h2[:],
                                            in1=csum_sb[:], op=OP.subtract)
                    t2w = scp.tile([P, R2E], BF16, tag="t2w")
                    nc.scalar.copy(t2w[:, 0:C], t2keep[:, t, 0:C])
                    nc.vector.tensor_copy(t2w[:, C : C + 2].bitcast(F32),
                                          t2keep[:, t, C : C + 1])
                    nc.sync.dma_start(d_t2s[t * P : t * P + rows, 0:R2E],
                                      t2w[:rows, :])
                    t2b = scp.tile([P, 1], BF16, tag="t2b")
                    nc.vector.tensor_copy(t2b[:], t2keep[:, t, R2E - 1 : R2E])
                    for c in tchunks_of(gi, t):
                        nc.tensor.matmul(
                            out=h2b[:, c : c + 1],
                            lhsT=seT[:, c * P : (c + 1) * P],
                            rhs=t2b[:], start=True, stop=True)
                nc.vector.tensor_copy(adst2e_sb[:, c0 : c0 + ct],
                                      h2b[:, 0:ct])

            pipe = []
            for gi in sched["groups"]:
                pipe.append(b_stage1(gi))
                if len(pipe) >= 2:
                    b_stage2(pipe[-2])
                if len(pipe) >= 3:
                    b_stage3(pipe[-3])
            b_stage2(pipe[-1])
            b_stage3(pipe[-2])
            b_stage3(pipe[-1])

        # w2s = exp(lrelu(a2src_own + a2dst_own)); uself2 = t2feat * w2s
        with tc.tile_pool(name="w2p", bufs=1) as w2p:
            a2o = w2p.tile([P, ntile], F32)
            nc.vector.tensor_tensor(
                out=a2o[:].rearrange("p (t o) -> p t o", o=1),
                in0=t2keep[:, :, C : C + 1],
                in1=t2keep[:, :, C + 1 : C + 2], op=OP.add)
            nc.scalar.activation(a2o[:], a2o[:], AF.Prelu, alpha=NEG_SLOPE)
            nc.scalar.activation(w2s_sb[:], a2o[:], AF.Exp)
            nc.vector.tensor_tensor(
                out=uself2_sb[:],
                in0=t2keep[:, :, 0:C],
                in1=w2s_sb[:].rearrange("p (t o) -> p t o", o=1)
                    .to_broadcast([P, ntile, C]),
                op=OP.mult)

        # ---- collective: share T2 slabs ----
        nc.gpsimd.collective_compute(
            "AllGather", OP.bypass, replica_groups=[list(range(ncore))],
            ins=[d_t2s[:, :]], outs=[d_t2f[:, :]])

        # ---- phase D: layer-2 aggregation ----
        with tc.tile_pool(name="g2", bufs=3) as g2p, \
             tc.tile_pool(name="sc2", bufs=3) as sc2, \
             tc.tile_pool(name="se2", bufs=2) as se2p, \
             tc.tile_pool(name="agg2", bufs=3, space="PSUM") as agg2p:
            for gi in sched["groups"]:
                c0, cl, chg, ct = gi["c0"], gi["clow"], gi["chigh"], gi["ct"]
                idx_t = sc2.tile([P, ct * 8], I16, tag="idx2")
                nc.sync.dma_start(idx_t[:], i_idx[:, c0 * 8 : (c0 + ct) * 8])
                dcol_t = sc2.tile([P, ct], F32, tag="dcol2")
                nc.sync.dma_start(dcol_t[:], i_dstcol[:, c0 : c0 + ct])
                G2 = g2p.tile([P, ct, ROW2], BF16, tag="G2")
                for (off, num, b0, b1_) in ((0, cl, 0, half),
                                            (cl, chg, half, n)):
                    if num == 0:
                        continue
                    nc.gpsimd.dma_gather(
                        out_ap=G2[:, off : off + num, :],
                        in_ap=d_t2f[b0:b1_, :],
                        idxs_ap=idx_t[:, off * 8 : (off + num) * 8],
                        num_idxs=num * P, num_idxs_reg=num * P,
                        elem_size=ROW2, single_packet=False)
                alpha = sc2.tile([P, ct], F32, tag="alpha2")
                nc.vector.tensor_tensor(
                    out=alpha[:].rearrange("p (c o) -> p c o", o=1),
                    in0=G2[:, :, C : C + 2].bitcast(F32),
                    in1=adst2e_sb[:, c0 : c0 + ct].rearrange(
                        "p (c o) -> p c o", o=1),
                    op=OP.add)
                nc.scalar.activation(alpha[:], alpha[:], AF.Prelu,
                                     alpha=NEG_SLOPE)
                nc.scalar.activation(
                    G2[:, :, C : C + 1],
                    alpha[:].rearrange("p (c o) -> p c o", o=1), AF.Exp)
                nc.vector.tensor_tensor(
                    out=G2[:, :, 0:C], in0=G2[:, :, 0:C],
                    in1=G2[:, :, C : C + 1].to_broadcast([P, ct, C]),
                    op=OP.mult)
                seA = se2p.tile([P, ct * P], BF16, tag="seA2")
                nc.vector.tensor_tensor(
                    out=seA[:].rearrange("p (c q) -> p c q", q=P),
                    in0=iota_rep[:, 0 : ct * P].rearrange(
                        "p (c q) -> p c q", q=P),
                    in1=dcol_t[:].rearrange(
                        "p (c o) -> p c o", o=1).to_broadcast([P, ct, P]),
                    op=OP.is_equal)
                for t in gi["tiles"]:
                    rows = sched["tile_rows"][t]
                    agg = agg2p.tile([P, C + 1], F32, tag="agg2")
                    tl, tlc = sched["tile_info"][t]["low"]
                    th, thc = sched["tile_info"][t]["high"]
                    tchunks = [tl - c0 + i_ for i_ in range(tlc)] + \
                              [th - c0 + i_ for i_ in range(thc)]
                    for j, c in enumerate(tchunks):
                        nc.tensor.matmul(
                            out=agg[:], lhsT=seA[:, c * P : (c + 1) * P],
                            rhs=G2[:, c, 0 : C + 1],
                            start=(j == 0), stop=(j == len(tchunks) - 1))
                    dn = sc2.tile([P, 1], F32, tag="dn2")
                    nc.vector.tensor_tensor(out=dn[:], in0=agg[:, C : C + 1],
                                            in1=w2s_sb[:, t : t + 1],
                                            op=OP.add)
                    rc = sc2.tile([P, 1], F32, tag="rc2")
                    nc.vector.reciprocal(rc[:], dn[:])
                    ob = sc2.tile([P, C], F32, tag="ob")
                    nc.vector.tensor_tensor(out=ob[:], in0=uself2_sb[:, t, :],
                                            in1=agg[:, 0:C], op=OP.add)
                    nc.vector.tensor_tensor(out=ob[:], in0=ob[:],
                                            in1=rc[:].to_broadcast([P, C]),
                                            op=OP.mult)
                    nc.vector.tensor_tensor(out=ob[:], in0=ob[:], in1=b2_sb[:],
                                            op=OP.add)
                    nc.sync.dma_start(o_out[t * P : t * P + rows, :],
                                      ob[:rows, :])

    nc.compile()
    return nc


# ---------------------------------------------------------------- entry
_CACHE = {}


def _run(inputs, p):
    x = np.asarray(inputs["x"], np.float32)
    ei = np.asarray(inputs["edge_index"])
    n, ncore = p["n"], p["ncore"]
    sh = n // ncore
    natile = math.ceil(n / P)
    npos = natile * P

    sched, percore, nodes_at, pos_of = build_schedule(
        np.asarray(ei[0], np.int64), np.asarray(ei[1], np.int64), p)
    key = (sched["nchunks"], tuple(sched["csched"].reshape(-1).tolist()))
    if key not in _CACHE:
        _CACHE.clear()
        _CACHE[key] = build_program(sched, p)
    nc = _CACHE[key]

    w1ev, r2ext, csum, b1p = prep_weights(
        np.asarray(inputs["W1"], np.float32),
        np.asarray(inputs["att_src1"], np.float32),
        np.asarray(inputs["att_dst1"], np.float32),
        np.asarray(inputs["bias1"], np.float32),
        np.asarray(inputs["W2"], np.float32),
        np.asarray(inputs["att_src2"], np.float32),
        np.asarray(inputs["att_dst2"], np.float32), p)
    b2 = np.asarray(inputs["bias2"], np.float32)[None, :]

    xpos = np.zeros((npos, p["in_ch"]), BF)
    xpos[0:n] = x[nodes_at].astype(BF)
    xT = np.ascontiguousarray(xpos[0:n].T)

    in_maps = []
    for k in range(ncore):
        in_maps.append(dict(
            xpos=xpos,
            xTmy=np.ascontiguousarray(xT[:, k * sh : (k + 1) * sh]),
            idx16=percore[k]["idx16"], dstcol=percore[k]["dstcol"],
            dstrow=percore[k]["dstrow"], w1ev=w1ev, r2ext=r2ext,
            csum=csum, bias1=b1p, bias2=b2))
    res = run_bass_kernel_spmd(nc, in_maps, core_ids=list(range(ncore)),
                               **p.get("run_kwargs", {}))
    out_pos = np.concatenate([res.results[k]["out"] for k in range(ncore)],
                             axis=0)
    out = np.empty_like(out_pos)
    out[nodes_at] = out_pos
    return out, res


def kernel(**inputs) -> np.ndarray:
    out, _ = _run(inputs, REAL)
    return out
